# revision 31
# baseline (speedup 1.0000x reference)
"""GQA attention (RoPE, causal) + output projection for Trainium2, 8 NeuronCores.

Problem: B=2, T=2048, HID=2048, NH=16 Q-heads, NKV=4 KV-heads, HD=128.
Sharding: tensor-parallel over the 4 KV-head groups (4 Q heads + 1 KV head per
group) x data-parallel over batch (2). Core c handles batch c//4, group c%4.
Each core computes its group's partial output y_g = A_g @ Wo[rows_g]; the
host unshards by summing the 4 row-parallel partials per batch.

All inputs are delivered bf16 from the host (x transposed and supertiled so
every DMA line is >=4KB contiguous), so no on-device dtype conversions or
staging copies exist anywhere.

Per-core device pipeline (all matmuls bf16, f32 accumulation in PSUM):
  A. Projections run per t-supertile (512 cols) into 1-bank PSUM tiles
     (8-deep ring): K, V, Q0..Q3 per supertile, so the x stream (one 2.1MB
     DMA group per supertile on the SyncE HWDGE queue) pipelines under the
     PE at 3.5x headroom and the final RoPE drain is only one 512-wide
     tile deep. RoPE rides cross-partition-base DVE reads fused with the
     sin multiply (sign folded into the host table); 1/sqrt(HD) is folded
     into the Q-side tables. V is copied to bf16 and XBAR-transposed to
     [t, d]. Weights/tables stream on ScalarE's HWDGE queue, needed-first.
  B. Per q-supertile: scores transposed ST[kv,q] for head PAIRS into
     2-bank PSUM tiles, so exp runs as one [128, 2*512] ScalarE ACTIVATE
     (halves the per-call pipeline-fill overhead); on diagonal supertiles
     all work is narrowed to the unmasked column range and one [128,128]
     triangle mask multiply zeroes the stragglers; AT[d,q] +=
     matmul(lhsT=V chunk, rhs=expST). Softmax sums: expST pairs accumulate
     on DVE (bf16); a ones-matmul per head produces partition-broadcast
     row sums; fast-approx reciprocal + DVE mul normalize into aT.
     The sums/normalize and the output projection of supertile qs are
     deferred into the later supertiles' kv loops (sum units drain before
     the first AV matmul so the single-buffered av PSUM tile can recycle;
     outproj units spread evenly over all remaining iterations), so the
     PE never head-of-line blocks on the DVE normalize chain and the exp
     bubbles are filled with outproj matmuls. Outproj units and the
     ones-matmuls allocate their PSUM from the same 2-bank pool as the
     score tiles (declared at bank 0 so stage B starts before stage A's
     last PSUM drain), keeping the whole stage inside 8 PSUM banks. The
     y copies alternate ScalarE/VectorE; y stores ride the idle SyncE
     queue as bf16 partials (the host unshard-sum is unchanged).
"""

import numpy as np
import ml_dtypes

import concourse.bass as bass
import concourse.mybir as mybir
import concourse.tile as tile
from concourse import bacc
from concourse.bass_utils import run_bass_kernel_spmd

B, T, HID = 2, 2048, 2048
NH, NKV = 16, 4
HD = 128
GROUPS = NH // NKV      # 4 q-heads per kv head
NQ = GROUPS             # q heads per core
QW = NQ * HD            # 512 q cols per core
P = 128
TB = T // P             # 16 t-blocks
HC = HID // P           # 16 hid chunks
QS = T // 512           # 4 q supertiles
KVC = T // P            # 16 kv chunks
TS = T // 512           # 4 t supertiles
ROPE_BASE = 10000.0

F32 = mybir.dt.float32
BF16 = mybir.dt.bfloat16
EXP = mybir.ActivationFunctionType.Exp


def build_nc():
    nc = bacc.Bacc("TRN2", target_bir_lowering=False, debug=False,
                   enable_asserts=False, num_devices=8)

    # x supertiled: [ts, hq, p, c, col] = xT[(hq*4+c)*128 + p, ts*512+col]
    x_d = nc.dram_tensor("xs", [TS, 4, P, 4, 512], BF16, kind="ExternalInput")
    wq_d = nc.dram_tensor("wq", [P, NQ, HC, HD], BF16, kind="ExternalInput")
    wk_d = nc.dram_tensor("wk", [P, HC, HD], BF16, kind="ExternalInput")
    wv_d = nc.dram_tensor("wv", [P, HC, HD], BF16, kind="ExternalInput")
    wo_d = nc.dram_tensor("wo", [P, NQ, HID], BF16, kind="ExternalInput")
    cosq_d = nc.dram_tensor("cosqT", [HD, T], BF16, kind="ExternalInput")
    sinq_d = nc.dram_tensor("sinqT", [HD, T], BF16, kind="ExternalInput")
    cosk_d = nc.dram_tensor("coskT", [HD, T], BF16, kind="ExternalInput")
    sink_d = nc.dram_tensor("sinkT", [HD, T], BF16, kind="ExternalInput")
    masks_d = nc.dram_tensor("masks", [P, P], BF16, kind="ExternalInput")
    y_d = nc.dram_tensor("y", [T, HID], BF16, kind="ExternalOutput")

    with tile.TileContext(nc) as tc:
        with tc.tile_pool(name="persist", bufs=1) as persist:
            # ---- persistent SBUF ----
            qT = persist.tile([P, NQ, T], BF16)        # (d, h, t)
            kT = persist.tile([P, T], BF16)            # (d, t)
            vnat = persist.tile([P, KVC, HD], BF16)    # (t, kvc, d)
            aT = persist.tile([P, NQ, T], BF16)        # (d, h, t)
            wq_s = persist.tile([P, NQ, HC, HD], BF16)
            wk_s = persist.tile([P, HC, HD], BF16)
            wv_s = persist.tile([P, HC, HD], BF16)
            wo_s = persist.tile([P, NQ, HID], BF16)
            cq_s = persist.tile([P, T], BF16)
            sq_s = persist.tile([P, T], BF16)
            ck_s = persist.tile([P, T], BF16)
            sk_s = persist.tile([P, T], BF16)
            masks_s = persist.tile([P, P], BF16)
            ones_s = persist.tile([P, P], BF16)

            # ---- stage A: projections + RoPE, per t-supertile ----
            with (
                tc.tile_pool(name="psA", bufs=1, space="PSUM") as psA,
                tc.tile_pool(name="stageA", bufs=1) as stageA,
            ):
                # weights/tables on the ScalarE HWDGE queue, ordered by the
                # PE-time each consumer first needs them; the first chunks
                # are split small so the PE unblocks at minimum latency
                nc.scalar.dma_start(wk_s[:, 0], wk_d.ap()[:, 0])
                nc.scalar.dma_start(
                    wk_s[:, 1:].rearrange("p hc d -> p (hc d)"),
                    wk_d.ap()[:, 1:].rearrange("p hc d -> p (hc d)"))
                nc.scalar.dma_start(
                    wv_s.rearrange("p hc d -> p (hc d)"),
                    wv_d.ap().rearrange("p hc d -> p (hc d)"))
                nc.scalar.dma_start(
                    wq_s[:, 0].rearrange("p hc d -> p (hc d)"),
                    wq_d.ap()[:, 0].rearrange("p hc d -> p (hc d)"))
                nc.scalar.dma_start(ck_s[:], cosk_d[:])
                nc.scalar.dma_start(sk_s[:], sink_d[:])
                for h in range(1, NQ):
                    nc.scalar.dma_start(
                        wq_s[:, h].rearrange("p hc d -> p (hc d)"),
                        wq_d.ap()[:, h].rearrange("p hc d -> p (hc d)"))
                nc.scalar.dma_start(cq_s[:], cosq_d[:])
                nc.scalar.dma_start(sq_s[:], sinq_d[:])
                nc.scalar.dma_start(masks_s[:], masks_d[:])

                def proj(xt, w_chunk):
                    ps = psA.tile([P, 512], F32, tag="ps", bufs=8)
                    for hc in range(HC):
                        nc.tensor.matmul(ps[:], w_chunk(hc), xt[:, hc],
                                         start=(hc == 0), stop=(hc == HC - 1))
                    return ps

                def rope(ps, cs, ss, out):
                    # rot-half fused with the sin multiply via
                    # cross-partition-base reads (sign folded into ss)
                    rot = stageA.tile([P, 512], F32, tag="rot", bufs=3,
                                      name="rot")
                    nc.vector.tensor_mul(rot[0:64, :], ps[64:128, :],
                                         ss[0:64, :])
                    nc.vector.tensor_mul(rot[64:128, :], ps[0:64, :],
                                         ss[64:128, :])
                    nc.vector.tensor_mul(out, ps[:], cs)
                    nc.vector.tensor_add(out, out, rot[:])

                for ts in range(TS):
                    t0 = ts * 512
                    xt = stageA.tile([P, HC, 512], BF16, tag="xts", bufs=3,
                                     name="xt")
                    if ts == 0:
                        # first hid chunk alone so the K projection's first
                        # matmul unblocks at minimum DMA latency
                        nc.sync.dma_start(xt[:, 0], x_d.ap()[0, 0, :, 0])
                        nc.sync.dma_start(xt[:, 1:4], x_d.ap()[0, 0, :, 1:])
                        for hq in range(1, 4):
                            nc.sync.dma_start(xt[:, hq * 4:(hq + 1) * 4, :],
                                              x_d.ap()[ts, hq])
                    else:
                        for hq in range(4):
                            nc.sync.dma_start(xt[:, hq * 4:(hq + 1) * 4, :],
                                              x_d.ap()[ts, hq])

                    ps = proj(xt, lambda hc: wk_s[:, hc])
                    rope(ps, ck_s[:, t0:t0 + 512], sk_s[:, t0:t0 + 512],
                         kT[:, t0:t0 + 512])

                    ps = proj(xt, lambda hc: wv_s[:, hc])
                    vtb = stageA.tile([P, 512], BF16, tag="vtb", bufs=2,
                                      name="vtb")
                    nc.scalar.copy(vtb[:], ps[:])
                    for j in range(4):
                        nc.sync.dma_start_transpose(
                            vnat[:, ts * 4 + j, :], vtb[:, j * P:(j + 1) * P])


                    for h in range(NQ):
                        ps = proj(xt, lambda hc: wq_s[:, h, hc])
                        rope(ps, cq_s[:, t0:t0 + 512], sq_s[:, t0:t0 + 512],
                             qT[:, h, t0:t0 + 512])

                # wo rides the GpSimd queue after all x stripes, off the
                # startup-critical window but well before the first outproj
                nc.gpsimd.dma_start(
                    wo_s.rearrange("p h c -> p (h c)"),
                    wo_d.ap().rearrange("p h c -> p (h c)"))

            # ---- stage B: attention + deferred sums/normalize/outproj ----
            nc.vector.memset(ones_s[:], 1.0)
            with (
                # psS first so its banks recycle stage A's early-drained ring
                # slots while psAv waits for the last RoPE reads
                tc.tile_pool(name="psS", bufs=2, space="PSUM") as psS,
                tc.tile_pool(name="psAv", bufs=1, space="PSUM") as psAv,
                tc.tile_pool(name="stageB", bufs=2) as stageB,
            ):
                def st_tile(name):
                    # 2-bank PSUM tiles shared by scores / ones-mm / outproj
                    return psS.tile([P, 2, 512], F32, tag="st", bufs=2,
                                    name=name)

                def sums_unit(qs, av, lacc, pi):
                    # row sums + reciprocal + normalize for head pair pi
                    q0 = qs * 512
                    lb = st_tile("lb")
                    for hl in range(2):
                        nc.tensor.matmul(lb[:, hl], ones_s[:], lacc[:, hl],
                                         start=True, stop=True)
                    rec = stageB.tile([P, 2, 512], F32, tag="rec", bufs=2,
                                      name="rec")
                    nc.vector.reciprocal_approx_fast(
                        rec.rearrange("p a b -> p (a b)"),
                        lb.rearrange("p a b -> p (a b)"))
                    for hl in range(2):
                        h = 2 * pi + hl
                        nc.vector.tensor_mul(aT[:, h, q0:q0 + 512],
                                             av[:, hl], rec[:, hl])

                def outproj_mms(tb, npair):
                    yp = st_tile("yp")
                    for cc in range(NQ):
                        for k in range(2):
                            ns = 2 * npair + k
                            nc.tensor.matmul(
                                yp[:, k], aT[:, cc, tb * P:(tb + 1) * P],
                                wo_s[:, cc, ns * 512:(ns + 1) * 512],
                                start=(cc == 0), stop=(cc == NQ - 1))
                    return yp

                def outproj_copyout(tb, npair, yp):
                    # issued two drain slots after the matmuls so neither
                    # engine head-of-line blocks waiting on the PE (a
                    # blocked ScalarE would push back the exp chain);
                    # halves split across ScalarE/VectorE so the yp banks
                    # recycle in half the time
                    y_sb = stageB.tile([P, 2, 512], BF16, tag="ysb", bufs=4,
                                       name="y_sb")
                    nc.scalar.copy(y_sb[:, 0], yp[:, 0])
                    nc.vector.tensor_copy(y_sb[:, 1], yp[:, 1])
                    nc.sync.dma_start(
                        y_d[tb * P:(tb + 1) * P,
                            npair * 1024:(npair + 1) * 1024],
                        y_sb.rearrange("p a b -> p (a b)"))

                # one pass per (q supertile, head pair): av double-buffers
                # across passes, so the normalize of pass p only gates the
                # av recycle of pass p+2
                pend_s = []      # sums unit of the previous pass
                pend_o = []      # outproj units of previous supertiles
                for qs in range(QS):
                    q0 = qs * 512
                    nkv = (qs + 1) * 4
                    for pi in range(2):
                        av = psAv.tile([P, 2, 512], F32, tag="av", bufs=2,
                                       name="av")
                        lacc = stageB.tile([P, 2, 512], BF16, bufs=4,
                                           tag="lacc", name="lacc")
                        # spread deferred outproj units at roughly half the
                        # queue per pass (each supertile's units ride the
                        # two passes of the next supertile)
                        n_po = len(pend_o)
                        drained = 0

                        def av_mms(prev):
                            kvp, c0p, pstp = prev
                            for hl in range(2):
                                nc.tensor.matmul(av[:, hl, c0p:],
                                                 vnat[:, kvp],
                                                 pstp[:, hl, c0p:],
                                                 start=(kvp == 0),
                                                 stop=(kvp == nkv - 1),
                                                 skip_group_check=True)

                        prev_av = []
                        for kvc in range(nkv):
                            o = kvc - 4 * qs
                            c0 = max(o, 0) * P
                            st_ps = st_tile("st_ps")
                            for hl in range(2):
                                h = 2 * pi + hl
                                nc.tensor.matmul(
                                    st_ps[:, hl, c0:],
                                    kT[:, kvc * P:(kvc + 1) * P],
                                    qT[:, h, q0 + c0:q0 + 512],
                                    start=True, stop=True)
                            pst = stageB.tile([P, 2, 512], BF16, tag="pst",
                                              bufs=10, name="pst")
                            nc.scalar.activation(pst[:, :, c0:],
                                                 st_ps[:, :, c0:], EXP)
                            if o >= 0:
                                for hl in range(2):
                                    nc.vector.tensor_mul(
                                        pst[:, hl, c0:c0 + P],
                                        pst[:, hl, c0:c0 + P], masks_s[:])
                            if kvc == 0:
                                nc.vector.tensor_copy(
                                    lacc.rearrange("p a b -> p (a b)"),
                                    pst.rearrange("p a b -> p (a b)"))
                            else:
                                nc.vector.tensor_add(
                                    lacc[:, :, c0:],
                                    lacc[:, :, c0:], pst[:, :, c0:])
                            # sums units must drain a pass ahead of the av
                            # ring reuse
                            if kvc == 0:
                                while pend_s:
                                    pend_s.pop(0)()
                            # two-iteration software pipeline: issue the AV
                            # matmuls two iterations behind the scores, so
                            # the exp chain has ~2 iterations of slack and
                            # the PE never waits on it even through queue
                            # jitter
                            if len(prev_av) == 2:
                                av_mms(prev_av.pop(0))
                            prev_av.append((kvc, c0, pst))
                            want = min(n_po, -(-n_po * (kvc + 1)
                                               // ((2 - pi) * nkv)))
                            # at most one matmul micro-unit per iteration
                            # (copy-outs ride along for free), so an
                            # outproj's copy-out lands a full iteration
                            # after its matmuls
                            while drained < want and pend_o:
                                kind = pend_o[0][0]
                                pend_o.pop(0)[1]()
                                drained += 1
                                if kind == "m":
                                    break
                        while prev_av:
                            av_mms(prev_av.pop(0))
                        pend_s.append((lambda qq, aa, ll, pp:
                                       lambda: sums_unit(qq, aa, ll, pp))
                                      (qs, av, lacc, pi))
                        if pi == 1:
                            ms, cs = [], []
                            for tb in range(4 * qs, 4 * qs + 4):
                                for np_ in range(2):
                                    def mk(t, n):
                                        box = {}

                                        def mms():
                                            box["yp"] = outproj_mms(t, n)

                                        def cp():
                                            outproj_copyout(t, n, box["yp"])
                                        return mms, cp
                                    u1, u2 = mk(tb, np_)
                                    ms.append(("m", u1))
                                    cs.append(("c", u2))
                            # stagger copy-outs two matmul slots behind
                            # their matmuls
                            for j, m in enumerate(ms):
                                pend_o.append(m)
                                if j >= 2:
                                    pend_o.append(cs[j - 2])
                            pend_o += cs[-2:]
                for unit in pend_s:
                    unit()
                while pend_o:
                    pend_o.pop(0)[1]()

    nc.compile()
    return nc


def make_tables():
    inv_freq = 1.0 / (ROPE_BASE ** (np.arange(0, HD, 2, dtype=np.float64) / HD))
    t = np.arange(T, dtype=np.float64)
    freqs = np.outer(t, inv_freq)
    emb = np.concatenate([freqs, freqs], axis=-1)        # [T, 128]
    cos = np.cos(emb)
    sin = np.sin(emb)
    sin_signed = sin.copy()
    sin_signed[:, :64] = -sin_signed[:, :64]
    scale = 1.0 / np.sqrt(HD)
    bf = ml_dtypes.bfloat16
    cosqT = np.ascontiguousarray((cos * scale).T).astype(bf)
    sinqT = np.ascontiguousarray((sin_signed * scale).T).astype(bf)
    coskT = np.ascontiguousarray(cos.T).astype(bf)
    sinkT = np.ascontiguousarray(sin_signed.T).astype(bf)
    return cosqT, sinqT, coskT, sinkT


def make_masks():
    # triangle mask [kv=128, q=128]: 1 where kv_row <= q_col
    j = np.arange(P)[None, :]
    i = np.arange(P)[:, None]
    return (i <= j).astype(ml_dtypes.bfloat16)


def make_in_maps(x, Wq, Wk, Wv, Wo):
    cosqT, sinqT, coskT, sinkT = make_tables()
    masks = make_masks()
    bf = ml_dtypes.bfloat16
    in_maps = []
    for c in range(8):
        b, g = c // 4, c % 4
        xT = x[b].T                                   # [HID, T]
        # [ts, hq, p, c, col] = xT[(hq*4+c)*128 + p, ts*512+col]
        xs = (xT.reshape(4, 4, P, TS, 512).transpose(3, 0, 2, 1, 4))
        in_maps.append({
            "xs": np.ascontiguousarray(xs).astype(bf),
            "wq": np.ascontiguousarray(
                Wq[:, g * QW:(g + 1) * QW].reshape(HC, P, NQ, HD)
                .transpose(1, 2, 0, 3)).astype(bf),
            "wk": np.ascontiguousarray(
                Wk[:, g * HD:(g + 1) * HD].reshape(HC, P, HD)
                .transpose(1, 0, 2)).astype(bf),
            "wv": np.ascontiguousarray(
                Wv[:, g * HD:(g + 1) * HD].reshape(HC, P, HD)
                .transpose(1, 0, 2)).astype(bf),
            "wo": np.ascontiguousarray(
                Wo[g * QW:(g + 1) * QW, :].reshape(NQ, P, HID)
                .transpose(1, 0, 2)).astype(bf),
            "cosqT": cosqT, "sinqT": sinqT, "coskT": coskT, "sinkT": sinkT,
            "masks": masks,
        })
    return in_maps


_NC_CACHE = None


def kernel(x, Wq, Wk, Wv, Wo, _trace=False, _tmpdir=None):
    global _NC_CACHE
    x = np.asarray(x, dtype=np.float32)
    Wq = np.asarray(Wq, dtype=np.float32)
    Wk = np.asarray(Wk, dtype=np.float32)
    Wv = np.asarray(Wv, dtype=np.float32)
    Wo = np.asarray(Wo, dtype=np.float32)

    if _NC_CACHE is None:
        _NC_CACHE = build_nc()
    nc = _NC_CACHE

    in_maps = make_in_maps(x, Wq, Wk, Wv, Wo)
    res = run_bass_kernel_spmd(nc, in_maps, core_ids=list(range(8)),
                               trace=_trace, tmpdir=_tmpdir)
    out = np.zeros((B, T, HID), dtype=np.float32)
    for c in range(8):
        out[c // 4] += res.results[c]["y"].astype(np.float32)
    if _trace:
        return out, res
    return out


# revision 32
# speedup vs baseline: 1.1488x; 1.1488x over previous
"""GQA attention (RoPE, causal) + output projection for Trainium2, 8 NeuronCores.

Problem: B=2, T=2048, HID=2048, NH=16 Q-heads, NKV=4 KV-heads, HD=128.
Sharding: tensor-parallel over the 4 KV-head groups (4 Q heads + 1 KV head per
group) x data-parallel over batch (2). Core c handles batch c//4, group c%4.
Each core computes its group's partial output y_g = A_g @ Wo[rows_g]; the
host unshards by summing the 4 row-parallel partials per batch.

All inputs are delivered bf16 from the host (x transposed and supertiled so
every DMA line is >=4KB contiguous), so no on-device dtype conversions or
staging copies exist anywhere.

Per-core device pipeline (all matmuls bf16, f32 accumulation in PSUM):
  A. Projections run per t-supertile (512 cols) into 1-bank PSUM tiles
     (8-deep ring): K, V, Q0..Q3 per supertile, so the x stream (one 2.1MB
     DMA group per supertile on the SyncE HWDGE queue) pipelines under the
     PE at 3.5x headroom and the final RoPE drain is only one 512-wide
     tile deep. RoPE rides cross-partition-base DVE reads fused with the
     sin multiply (sign folded into the host table); 1/sqrt(HD) is folded
     into the Q-side tables. V is copied to bf16 and XBAR-transposed to
     [t, d]. Weights/tables stream on ScalarE's HWDGE queue, needed-first.
  B. Per q-supertile: scores transposed ST[kv,q] for head PAIRS into
     2-bank PSUM tiles, so exp runs as one [128, 2*512] ScalarE ACTIVATE
     (halves the per-call pipeline-fill overhead); on diagonal supertiles
     all work is narrowed to the unmasked column range and one [128,128]
     triangle mask multiply zeroes the stragglers; AT[d,q] +=
     matmul(lhsT=V chunk, rhs=expST). Softmax sums: expST pairs accumulate
     on DVE (bf16); a ones-matmul per head produces partition-broadcast
     row sums; fast-approx reciprocal + DVE mul normalize into aT.
     The sums/normalize and the output projection of supertile qs are
     deferred into the later supertiles' kv loops (sum units drain before
     the first AV matmul so the single-buffered av PSUM tile can recycle;
     outproj units spread evenly over all remaining iterations), so the
     PE never head-of-line blocks on the DVE normalize chain and the exp
     bubbles are filled with outproj matmuls. Outproj units and the
     ones-matmuls allocate their PSUM from the same 2-bank pool as the
     score tiles (declared at bank 0 so stage B starts before stage A's
     last PSUM drain), keeping the whole stage inside 8 PSUM banks. The
     y copies alternate ScalarE/VectorE; y stores ride the idle SyncE
     queue as bf16 partials (the host unshard-sum is unchanged).
"""

import numpy as np
import ml_dtypes

import concourse.bass as bass
import concourse.mybir as mybir
import concourse.tile as tile
from concourse import bacc
from concourse.bass_utils import run_bass_kernel_spmd

B, T, HID = 2, 2048, 2048
NH, NKV = 16, 4
HD = 128
GROUPS = NH // NKV      # 4 q-heads per kv head
NQ = GROUPS             # q heads per core
QW = NQ * HD            # 512 q cols per core
P = 128
TB = T // P             # 16 t-blocks
HC = HID // P           # 16 hid chunks
QS = T // 512           # 4 q supertiles
KVC = T // P            # 16 kv chunks
TS = T // 512           # 4 t supertiles
ROPE_BASE = 10000.0

F32 = mybir.dt.float32
BF16 = mybir.dt.bfloat16
EXP = mybir.ActivationFunctionType.Exp


def build_nc():
    nc = bacc.Bacc("TRN2", target_bir_lowering=False, debug=False,
                   enable_asserts=False, num_devices=8)

    # x supertiled: [ts, hq, p, c, col] = xT[(hq*4+c)*128 + p, ts*512+col]
    x_d = nc.dram_tensor("xs", [TS, 4, P, 4, 512], BF16, kind="ExternalInput")
    wq_d = nc.dram_tensor("wq", [P, NQ, HC, HD], BF16, kind="ExternalInput")
    wk_d = nc.dram_tensor("wk", [P, HC, HD], BF16, kind="ExternalInput")
    wv_d = nc.dram_tensor("wv", [P, HC, HD], BF16, kind="ExternalInput")
    wo_d = nc.dram_tensor("wo", [P, NQ, HID], BF16, kind="ExternalInput")
    cosq_d = nc.dram_tensor("cosqT", [HD, T], BF16, kind="ExternalInput")
    sinq_d = nc.dram_tensor("sinqT", [HD, T], BF16, kind="ExternalInput")
    cosk_d = nc.dram_tensor("coskT", [HD, T], BF16, kind="ExternalInput")
    sink_d = nc.dram_tensor("sinkT", [HD, T], BF16, kind="ExternalInput")
    masks_d = nc.dram_tensor("masks", [P, P], BF16, kind="ExternalInput")
    y_d = nc.dram_tensor("y", [T, HID], BF16, kind="ExternalOutput")

    with tile.TileContext(nc) as tc:
        with tc.tile_pool(name="persist", bufs=1) as persist:
            # ---- persistent SBUF ----
            qT = persist.tile([P, NQ, T], BF16)        # (d, h, t)
            kT = persist.tile([P, T], BF16)            # (d, t)
            vnat = persist.tile([P, KVC, HD], BF16)    # (t, kvc, d)
            aT = persist.tile([P, NQ, T], BF16)        # (d, h, t)
            wq_s = persist.tile([P, NQ, HC, HD], BF16)
            wk_s = persist.tile([P, HC, HD], BF16)
            wv_s = persist.tile([P, HC, HD], BF16)
            wo_s = persist.tile([P, NQ, HID], BF16)
            cq_s = persist.tile([P, T], BF16)
            sq_s = persist.tile([P, T], BF16)
            ck_s = persist.tile([P, T], BF16)
            sk_s = persist.tile([P, T], BF16)
            masks_s = persist.tile([P, P], BF16)
            ones_s = persist.tile([P, P], BF16)

            # ---- stage A: projections + RoPE, per t-supertile ----
            with (
                tc.tile_pool(name="psA", bufs=1, space="PSUM") as psA,
                tc.tile_pool(name="stageA", bufs=1) as stageA,
            ):
                # weights/tables on the ScalarE HWDGE queue, ordered by the
                # PE-time each consumer first needs them; the first chunks
                # are split small so the PE unblocks at minimum latency
                nc.scalar.dma_start(wk_s[:, 0], wk_d.ap()[:, 0])
                nc.scalar.dma_start(
                    wk_s[:, 1:].rearrange("p hc d -> p (hc d)"),
                    wk_d.ap()[:, 1:].rearrange("p hc d -> p (hc d)"))
                nc.scalar.dma_start(
                    wv_s.rearrange("p hc d -> p (hc d)"),
                    wv_d.ap().rearrange("p hc d -> p (hc d)"))
                nc.scalar.dma_start(
                    wq_s[:, 0].rearrange("p hc d -> p (hc d)"),
                    wq_d.ap()[:, 0].rearrange("p hc d -> p (hc d)"))
                nc.scalar.dma_start(ck_s[:], cosk_d[:])
                nc.scalar.dma_start(sk_s[:], sink_d[:])
                for h in range(1, NQ):
                    nc.scalar.dma_start(
                        wq_s[:, h].rearrange("p hc d -> p (hc d)"),
                        wq_d.ap()[:, h].rearrange("p hc d -> p (hc d)"))
                nc.scalar.dma_start(cq_s[:], cosq_d[:])
                nc.scalar.dma_start(sq_s[:], sinq_d[:])
                nc.scalar.dma_start(masks_s[:], masks_d[:])

                def proj(xt, w_chunk):
                    ps = psA.tile([P, 512], F32, tag="ps", bufs=8)
                    for hc in range(HC):
                        nc.tensor.matmul(ps[:], w_chunk(hc), xt[:, hc],
                                         start=(hc == 0), stop=(hc == HC - 1))
                    return ps

                def rope(ps, cs, ss, out):
                    # rot-half fused with the sin multiply via
                    # cross-partition-base reads (sign folded into ss)
                    rot = stageA.tile([P, 512], F32, tag="rot", bufs=3,
                                      name="rot")
                    nc.vector.tensor_mul(rot[0:64, :], ps[64:128, :],
                                         ss[0:64, :])
                    nc.vector.tensor_mul(rot[64:128, :], ps[0:64, :],
                                         ss[64:128, :])
                    nc.vector.tensor_mul(out, ps[:], cs)
                    nc.vector.tensor_add(out, out, rot[:])

                for ts in range(TS):
                    t0 = ts * 512
                    xt = stageA.tile([P, HC, 512], BF16, tag="xts", bufs=3,
                                     name="xt")
                    if ts == 0:
                        # first hid chunk alone so the K projection's first
                        # matmul unblocks at minimum DMA latency
                        nc.sync.dma_start(xt[:, 0], x_d.ap()[0, 0, :, 0])
                        nc.sync.dma_start(xt[:, 1:4], x_d.ap()[0, 0, :, 1:])
                        for hq in range(1, 4):
                            nc.sync.dma_start(xt[:, hq * 4:(hq + 1) * 4, :],
                                              x_d.ap()[ts, hq])
                    else:
                        for hq in range(4):
                            nc.sync.dma_start(xt[:, hq * 4:(hq + 1) * 4, :],
                                              x_d.ap()[ts, hq])

                    ps = proj(xt, lambda hc: wk_s[:, hc])
                    rope(ps, ck_s[:, t0:t0 + 512], sk_s[:, t0:t0 + 512],
                         kT[:, t0:t0 + 512])

                    ps = proj(xt, lambda hc: wv_s[:, hc])
                    vtb = stageA.tile([P, 512], BF16, tag="vtb", bufs=2,
                                      name="vtb")
                    nc.scalar.copy(vtb[:], ps[:])
                    for j in range(4):
                        nc.sync.dma_start_transpose(
                            vnat[:, ts * 4 + j, :], vtb[:, j * P:(j + 1) * P])


                    for h in range(NQ):
                        ps = proj(xt, lambda hc: wq_s[:, h, hc])
                        rope(ps, cq_s[:, t0:t0 + 512], sq_s[:, t0:t0 + 512],
                             qT[:, h, t0:t0 + 512])

                # wo rides the GpSimd queue after all x stripes, off the
                # startup-critical window but well before the first outproj
                nc.gpsimd.dma_start(
                    wo_s.rearrange("p h c -> p (h c)"),
                    wo_d.ap().rearrange("p h c -> p (h c)"))

            # ---- stage B: attention + deferred sums/normalize/outproj ----
            nc.vector.memset(ones_s[:], 1.0)
            with (
                # psS first so its banks recycle stage A's early-drained ring
                # slots while psAv waits for the last RoPE reads
                tc.tile_pool(name="psS", bufs=2, space="PSUM") as psS,
                tc.tile_pool(name="psAv", bufs=1, space="PSUM") as psAv,
                tc.tile_pool(name="stageB", bufs=2) as stageB,
            ):
                def st_tile(name):
                    # 2-bank PSUM tiles shared by scores / ones-mm / outproj
                    return psS.tile([P, 2, 512], F32, tag="st", bufs=2,
                                    name=name)

                def sums_unit(qs, av, lacc, pi):
                    # row sums + reciprocal + normalize for head pair pi
                    q0 = qs * 512
                    lb = st_tile("lb")
                    for hl in range(2):
                        nc.tensor.matmul(lb[:, hl], ones_s[:], lacc[:, hl],
                                         start=True, stop=True)
                    rec = stageB.tile([P, 2, 512], F32, tag="rec", bufs=2,
                                      name="rec")
                    nc.vector.reciprocal_approx_fast(
                        rec.rearrange("p a b -> p (a b)"),
                        lb.rearrange("p a b -> p (a b)"))
                    for hl in range(2):
                        h = 2 * pi + hl
                        nc.vector.tensor_mul(aT[:, h, q0:q0 + 512],
                                             av[:, hl], rec[:, hl])

                def outproj_mms(tb, npair):
                    yp = st_tile("yp")
                    for cc in range(NQ):
                        for k in range(2):
                            ns = 2 * npair + k
                            nc.tensor.matmul(
                                yp[:, k], aT[:, cc, tb * P:(tb + 1) * P],
                                wo_s[:, cc, ns * 512:(ns + 1) * 512],
                                start=(cc == 0), stop=(cc == NQ - 1))
                    return yp

                def outproj_copyout(tb, npair, yp):
                    # issued two drain slots after the matmuls so neither
                    # engine head-of-line blocks waiting on the PE (a
                    # blocked ScalarE would push back the exp chain);
                    # halves split across ScalarE/VectorE so the yp banks
                    # recycle in half the time
                    y_sb = stageB.tile([P, 2, 512], BF16, tag="ysb", bufs=4,
                                       name="y_sb")
                    nc.scalar.copy(y_sb[:, 0], yp[:, 0])
                    nc.vector.tensor_copy(y_sb[:, 1], yp[:, 1])
                    nc.sync.dma_start(
                        y_d[tb * P:(tb + 1) * P,
                            npair * 1024:(npair + 1) * 1024],
                        y_sb.rearrange("p a b -> p (a b)"))

                # one pass per (q supertile, head pair): av double-buffers
                # across passes, so the normalize of pass p only gates the
                # av recycle of pass p+2
                pend_s = []      # sums unit of the previous pass
                pend_o = []      # outproj units of previous supertiles
                for qs in range(QS):
                    q0 = qs * 512
                    nkv = (qs + 1) * 4
                    for pi in range(2):
                        av = psAv.tile([P, 2, 512], F32, tag="av", bufs=2,
                                       name="av")
                        lacc = stageB.tile([P, 2, 512], BF16, bufs=4,
                                           tag="lacc", name="lacc")
                        # spread deferred outproj units at roughly half the
                        # queue per pass (each supertile's units ride the
                        # two passes of the next supertile)
                        n_po = len(pend_o)
                        drained = 0

                        def av_mms(prev):
                            kvp, c0p, pstp = prev
                            for hl in range(2):
                                nc.tensor.matmul(av[:, hl, c0p:],
                                                 vnat[:, kvp],
                                                 pstp[:, hl, c0p:],
                                                 start=(kvp == 0),
                                                 stop=(kvp == nkv - 1),
                                                 skip_group_check=True)

                        prev_av = []
                        for kvc in range(nkv):
                            o = kvc - 4 * qs
                            c0 = max(o, 0) * P
                            st_ps = st_tile("st_ps")
                            for hl in range(2):
                                h = 2 * pi + hl
                                nc.tensor.matmul(
                                    st_ps[:, hl, c0:],
                                    kT[:, kvc * P:(kvc + 1) * P],
                                    qT[:, h, q0 + c0:q0 + 512],
                                    start=True, stop=True)
                            pst = stageB.tile([P, 2, 512], BF16, tag="pst",
                                              bufs=10, name="pst")
                            nc.scalar.activation(pst[:, :, c0:],
                                                 st_ps[:, :, c0:], EXP)
                            if o >= 0:
                                for hl in range(2):
                                    nc.vector.tensor_mul(
                                        pst[:, hl, c0:c0 + P],
                                        pst[:, hl, c0:c0 + P], masks_s[:])
                            if kvc == 0:
                                nc.vector.tensor_copy(
                                    lacc.rearrange("p a b -> p (a b)"),
                                    pst.rearrange("p a b -> p (a b)"))
                            else:
                                nc.vector.tensor_add(
                                    lacc[:, :, c0:],
                                    lacc[:, :, c0:], pst[:, :, c0:])
                            # sums units must drain a pass ahead of the av
                            # ring reuse
                            if kvc == 0:
                                while pend_s:
                                    pend_s.pop(0)()
                            # one-iteration software pipeline: issue the AV
                            # matmuls of the PREVIOUS iteration, whose exp
                            # has already drained -- the PE never waits on
                            # the current exp
                            if len(prev_av) == 1:
                                av_mms(prev_av.pop(0))
                            prev_av.append((kvc, c0, pst))
                            want = min(n_po, -(-n_po * (kvc + 1)
                                               // ((2 - pi) * nkv)))
                            # at most one matmul micro-unit per iteration
                            # (copy-outs ride along for free), so an
                            # outproj's copy-out lands a full iteration
                            # after its matmuls
                            while drained < want and pend_o:
                                kind = pend_o[0][0]
                                pend_o.pop(0)[1]()
                                drained += 1
                                if kind == "m":
                                    break
                        while prev_av:
                            av_mms(prev_av.pop(0))
                        pend_s.append((lambda qq, aa, ll, pp:
                                       lambda: sums_unit(qq, aa, ll, pp))
                                      (qs, av, lacc, pi))
                        if pi == 1:
                            ms, cs = [], []
                            for tb in range(4 * qs, 4 * qs + 4):
                                for np_ in range(2):
                                    def mk(t, n):
                                        box = {}

                                        def mms():
                                            box["yp"] = outproj_mms(t, n)

                                        def cp():
                                            outproj_copyout(t, n, box["yp"])
                                        return mms, cp
                                    u1, u2 = mk(tb, np_)
                                    ms.append(("m", u1))
                                    cs.append(("c", u2))
                            # stagger copy-outs two matmul slots behind
                            # their matmuls
                            for j, m in enumerate(ms):
                                pend_o.append(m)
                                if j >= 2:
                                    pend_o.append(cs[j - 2])
                            pend_o += cs[-2:]
                for unit in pend_s:
                    unit()
                while pend_o:
                    pend_o.pop(0)[1]()

    nc.compile()
    return nc


def make_tables():
    inv_freq = 1.0 / (ROPE_BASE ** (np.arange(0, HD, 2, dtype=np.float64) / HD))
    t = np.arange(T, dtype=np.float64)
    freqs = np.outer(t, inv_freq)
    emb = np.concatenate([freqs, freqs], axis=-1)        # [T, 128]
    cos = np.cos(emb)
    sin = np.sin(emb)
    sin_signed = sin.copy()
    sin_signed[:, :64] = -sin_signed[:, :64]
    scale = 1.0 / np.sqrt(HD)
    bf = ml_dtypes.bfloat16
    cosqT = np.ascontiguousarray((cos * scale).T).astype(bf)
    sinqT = np.ascontiguousarray((sin_signed * scale).T).astype(bf)
    coskT = np.ascontiguousarray(cos.T).astype(bf)
    sinkT = np.ascontiguousarray(sin_signed.T).astype(bf)
    return cosqT, sinqT, coskT, sinkT


def make_masks():
    # triangle mask [kv=128, q=128]: 1 where kv_row <= q_col
    j = np.arange(P)[None, :]
    i = np.arange(P)[:, None]
    return (i <= j).astype(ml_dtypes.bfloat16)


def make_in_maps(x, Wq, Wk, Wv, Wo):
    cosqT, sinqT, coskT, sinkT = make_tables()
    masks = make_masks()
    bf = ml_dtypes.bfloat16
    in_maps = []
    for c in range(8):
        b, g = c // 4, c % 4
        xT = x[b].T                                   # [HID, T]
        # [ts, hq, p, c, col] = xT[(hq*4+c)*128 + p, ts*512+col]
        xs = (xT.reshape(4, 4, P, TS, 512).transpose(3, 0, 2, 1, 4))
        in_maps.append({
            "xs": np.ascontiguousarray(xs).astype(bf),
            "wq": np.ascontiguousarray(
                Wq[:, g * QW:(g + 1) * QW].reshape(HC, P, NQ, HD)
                .transpose(1, 2, 0, 3)).astype(bf),
            "wk": np.ascontiguousarray(
                Wk[:, g * HD:(g + 1) * HD].reshape(HC, P, HD)
                .transpose(1, 0, 2)).astype(bf),
            "wv": np.ascontiguousarray(
                Wv[:, g * HD:(g + 1) * HD].reshape(HC, P, HD)
                .transpose(1, 0, 2)).astype(bf),
            "wo": np.ascontiguousarray(
                Wo[g * QW:(g + 1) * QW, :].reshape(NQ, P, HID)
                .transpose(1, 0, 2)).astype(bf),
            "cosqT": cosqT, "sinqT": sinqT, "coskT": coskT, "sinkT": sinkT,
            "masks": masks,
        })
    return in_maps


_NC_CACHE = None


def kernel(x, Wq, Wk, Wv, Wo, _trace=False, _tmpdir=None):
    global _NC_CACHE
    x = np.asarray(x, dtype=np.float32)
    Wq = np.asarray(Wq, dtype=np.float32)
    Wk = np.asarray(Wk, dtype=np.float32)
    Wv = np.asarray(Wv, dtype=np.float32)
    Wo = np.asarray(Wo, dtype=np.float32)

    if _NC_CACHE is None:
        _NC_CACHE = build_nc()
    nc = _NC_CACHE

    in_maps = make_in_maps(x, Wq, Wk, Wv, Wo)
    res = run_bass_kernel_spmd(nc, in_maps, core_ids=list(range(8)),
                               trace=_trace, tmpdir=_tmpdir)
    out = np.zeros((B, T, HID), dtype=np.float32)
    for c in range(8):
        out[c // 4] += res.results[c]["y"].astype(np.float32)
    if _trace:
        return out, res
    return out
